# revision 17
# baseline (speedup 1.0000x reference)
"""Trainium2 Bass kernel for the AxialShift block (4x96x256x256, fp32).

Self-contained: builds an 8-core SPMD Bass program, compiles it once,
and runs it via run_bass_kernel_spmd.

Sharding: each core runs S=2 streams; stream s of core k handles a
quarter-sample slab (64 rows) of sample 2s + k//4 (4 cores share a
sample; GroupNorm(1 group) partials cross cores via 8-byte AllReduces).

Per-stream pipeline (x read once as fp16, out written once as fp16):
  A   : conv1 as one full-K fp16 matmul per 512-px unit over the 66-row
        halo frame; PSUM evacuation to the fp16 frame fused with the
        per-channel GroupNorm-1 sums via tensor_scalar accum_out
        (DVE/ACT split); sum-of-squares from the fp16 frame via
        scalar_tensor_tensor accum; per-channel partials collapsed with
        a ones-matmul; 8-byte AllReduce (result prefetched on the
        gpsimd queue so the wait does not block other engines).
  B   : GN1 scale/bias + erf-Gelu applied lazily in place (ACT); the 4
        axial-shift branch convs as 12 chunk-masked K=96 matmuls per
        512-px tile, two tiles in flight across all 8 PSUM banks
        (shifts are free-dim offsets into the padded frame); branch
        bias folded into the Gelu activation bias (N=2048 calls);
        branch sums split gpsimd/DVE, branch total + GroupNorm-2
        partials fused via scalar_tensor_tensor accum; second
        AllReduce.
  C   : GroupNorm-2 folded into conv3 weights on device (1/std from a
        DVE Newton rsqrt, avoiding ACT Sqrt table switches); one
        full-K matmul per tile alternating two PSUM rings; bias-add
        drain alternating DVE/ACT; fp16 output DMA.
"""
import sys

sys.path.insert(0, "/opt/trn_rl_repo")

import numpy as np

import concourse.bass as bass
import concourse.bacc as bacc
import concourse.tile as tile
from concourse import mybir

F32 = mybir.dt.float32
F16 = mybir.dt.float16

C = 96
M = 128           # weight free dim padded to 128 (FWL)
H = 256
W = 256
B = 4
WP = W + 2
N_CORES = 8
S = 2             # streams per core
ROWS_SLAB = H * B // (N_CORES * S)       # 64
RF = ROWS_SLAB + 2                       # 66 frame rows
NPAIR = ROWS_SLAB // 4                   # 16 tile-pairs (tile = 2 rows)
EPS = 1e-5
NPX = ROWS_SLAB * W                      # true pixels per stream
INV_N = 1.0 / (C * H * W)                # per-sample GroupNorm 1/N
V1MID = 0.337                            # measured conv1-output variance
V2MID = 0.2363
ROWTILE = False    # PE row-tiled K=32 matmuls (True) vs full-K masked blocks                           # measured branch-sum variance
AF = mybir.ActivationFunctionType
ALU = mybir.AluOpType
AX = mybir.AxisListType

# (dh, dw) read offsets per chunk j=0,1,2 (s_j = -1, 0, +1):
BR_LR = [(0, 1), (0, 0), (0, -1)]
BR_LDIAG = [(1, 1), (0, 0), (-1, -1)]
BR_TD = [(1, 0), (0, 0), (-1, 0)]
BR_RDIAG = [(1, -1), (0, 0), (-1, 1)]


def _bcast(ap, nparts):
    return bass.AP(tensor=ap.tensor, offset=ap.offset,
                   ap=[[0, nparts]] + list(ap.ap[1:]))


def _bcast_free(ap, dims):
    """Broadcast a [C,1] column over extra 0-stride free dims."""
    return bass.AP(tensor=ap.tensor, offset=ap.offset,
                   ap=[list(ap.ap[0])] + [[0, d] for d in dims])


class _Stream:
    """Per-stream state; stages are emitted by the orchestrator."""

    def __init__(self, nc, tc, pools, groups, io, s):
        self.nc, self.tc, self.s = nc, tc, s
        self.p = pools
        self.groups = groups
        self.io = io
        con = pools["consts"]
        big = pools["big"]
        self.xact = big.tile([C, RF, WP], F16, name=f"xact{s}")
        self.opre = big.tile([C, ROWS_SLAB * W], F16, name=f"opre{s}")
        self.scol = con.tile([C, 10], F32, name=f"sc_{s}")
        self.sqcol = con.tile([C, 9], F32, name=f"sq_{s}")
        self.junkc = con.tile([C, 2], F32, name=f"jc_{s}")
        self.sodcol = con.tile([C, NPAIR], F32, name=f"so_{s}")
        self.sq2col = con.tile([C, NPAIR], F32, name=f"s2_{s}")
        dram = pools["dram"]
        self.d1i = dram.tile([1, 2], F32, name=f"d1i{s}")
        self.d1o = dram.tile([1, 2], F32, name=f"d1o{s}")
        self.d2i = dram.tile([1, 2], F32, name=f"d2i{s}")
        self.d2o = dram.tile([1, 2], F32, name=f"d2o{s}")

    # ---------------- helpers ----------------
    def _rsqrt(self, vpe, mid, tag):
        """1/sqrt(vpe) on [C,1] via linear seed + 3 Newton iters (DVE)."""
        nc = self.nc
        con = self.p["consts"]
        rm = mid ** -0.5
        bcoef = -0.4764 * rm / mid
        acoef = 1.5768 * rm
        vh = con.tile([C, 1], F32, name=f"vh{tag}_{self.s}")
        nc.vector.tensor_scalar_mul(out=vh[:], in0=vpe, scalar1=0.5)
        r = con.tile([C, 1], F32, name=f"nr{tag}_{self.s}")
        nc.vector.tensor_scalar(out=r[:], in0=vpe, scalar1=bcoef,
                                scalar2=acoef, op0=ALU.mult, op1=ALU.add)
        u = con.tile([C, 1], F32, name=f"nu{tag}_{self.s}")
        w = con.tile([C, 1], F32, name=f"nw{tag}_{self.s}")
        for _ in range(3):
            nc.vector.tensor_mul(out=u[:], in0=r[:], in1=r[:])
            nc.vector.tensor_mul(out=w[:], in0=u[:], in1=vh[:])
            nc.vector.tensor_scalar(out=w[:], in0=w[:], scalar1=-1.0,
                                    scalar2=1.5, op0=ALU.mult, op1=ALU.add)
            nc.vector.tensor_mul(out=r[:], in0=r[:], in1=w[:])
        return r

    def _evac(self, pt_ap, out_ap, acc_ap, eng="v"):
        """PSUM->fp16 copy fused with per-channel sum (DVE or ACT)."""
        nc = self.nc
        if eng == "s":
            nc.scalar.activation(out=out_ap, in_=pt_ap, func=AF.Copy,
                                 bias=0.0, accum_out=acc_ap)
        else:
            nc.vector.tensor_scalar(out=out_ap, in0=pt_ap, scalar1=0.0,
                                    scalar2=None, op0=ALU.add, op1=ALU.add,
                                    accum_out=acc_ap)

    # ---------------- phase A ----------------
    def stage_a(self, psa):
        nc, s = self.nc, self.s
        xin = self.p["xin"]
        xs = self.io["xs"][s]
        w1t = self.p["w1t"]
        nc.vector.memset(self.xact[:, :, 0:1], 0.0)
        nc.vector.memset(self.xact[:, :, WP - 1:WP], 0.0)
        for b in range(9):
            r0 = 8 * b
            nr = 8 if b < 8 else 2
            xt = xin.tile([C, 8, W], F16, tag="xt")
            nc.sync.dma_start(out=xt[:, 0:nr, :], in_=xs[:, r0:r0 + nr, :])
            pt = psa.tile([M, 4, 512], F32, tag="pta")
            for q in range(nr // 2):
                if ROWTILE:
                    for j in range(3):
                        nc.tensor.matmul(
                            out=pt[:, q, :],
                            lhsT=w1t[32 * j:32 * (j + 1), :],
                            rhs=xt[32 * j:32 * (j + 1), 2 * q:2 * q + 2, :],
                            start=(j == 0), stop=(j == 2))
                else:
                    nc.tensor.matmul(out=pt[:, q, :], lhsT=w1t[:],
                                     rhs=xt[:, 2 * q:2 * q + 2, :],
                                     start=True, stop=True)
            if b == 0:
                self._evac(pt[0:C, 0, 0:256], self.xact[0:C, 0, 1:W + 1],
                           self.junkc[:, 0:1])
                self._evac(pt[0:C, 0, 256:512], self.xact[0:C, 1, 1:W + 1],
                           self.scol[:, 9:10])
                self._evac(
                    pt[0:C, 1:4, :].rearrange("p k (r w) -> p k r w", w=W),
                    self.xact[0:C, 2:8, 1:W + 1].rearrange(
                        "p (k r) w -> p k r w", r=2),
                    self.scol[:, 0:1])
                sq_in = self.xact[0:C, 1:8, 0:WP]
            elif b == 8:
                self._evac(pt[0:C, 0, 0:256], self.xact[0:C, 64, 1:W + 1],
                           self.scol[:, 8:9])
                self._evac(pt[0:C, 0, 256:512], self.xact[0:C, 65, 1:W + 1],
                           self.junkc[:, 1:2])
                sq_in = self.xact[0:C, 64:65, 0:WP]
            else:
                self._evac(
                    pt[0:C, :, :].rearrange("p k (r w) -> p k r w", w=W),
                    self.xact[0:C, r0:r0 + 8, 1:W + 1].rearrange(
                        "p (k r) w -> p k r w", r=2),
                    self.scol[:, b:b + 1],
                    eng="v")
                sq_in = self.xact[0:C, r0:r0 + 8, 0:WP]
            ja = self.p["tmp"].tile([C, 8, WP], F16, tag="ja")
            nc.scalar.activation(out=ja[:, 0:sq_in.shape[1], :], in_=sq_in,
                                 func=AF.Square, bias=0.0,
                                 accum_out=self.sqcol[:, b:b + 1])
        # aggregate per-channel partials, fold conv1 bias analytically
        con = self.p["consts"]
        cols = self.p["cols"]
        s1 = con.tile([C, 1], F32, name=f"s1_{s}")
        nc.vector.tensor_reduce(out=s1[:], in_=self.scol[:], axis=AX.X,
                                op=ALU.add)
        s2 = con.tile([C, 1], F32, name=f"s2s_{s}")
        nc.vector.tensor_reduce(out=s2[:], in_=self.sqcol[:], axis=AX.X,
                                op=ALU.add)
        tb = con.tile([C, 1], F32, name=f"tb_{s}")
        nc.vector.tensor_scalar_mul(out=tb[:], in0=cols[:, 0:1],
                                    scalar1=float(NPX))
        pack = con.tile([C, 2], F32, name=f"pk1_{s}")
        nc.vector.tensor_add(out=pack[:, 0:1], in0=s1[:], in1=tb[:])
        q = con.tile([C, 1], F32, name=f"q_{s}")
        nc.vector.tensor_scalar_mul(out=q[:], in0=s1[:], scalar1=2.0)
        nc.vector.tensor_add(out=q[:], in0=q[:], in1=tb[:])
        nc.vector.tensor_mul(out=q[:], in0=q[:], in1=cols[:, 0:1])
        nc.vector.tensor_add(out=pack[:, 1:2], in0=s2[:], in1=q[:])
        spt = psa.tile([M, 4, 512], F32, tag="pta")
        nc.tensor.matmul(out=spt[0:1, 0, 0:2], lhsT=self.p["ones96"][:],
                         rhs=pack[:], start=True, stop=True)
        ar_in = con.tile([1, 2], F32, name=f"ar1i_{s}")
        nc.vector.tensor_copy(out=ar_in[:], in_=spt[0:1, 0, 0:2])
        nc.sync.dma_start(out=self.d1i[:], in_=ar_in[:])
        nc.gpsimd.collective_compute(
            "AllReduce", ALU.add, replica_groups=self.groups,
            ins=[self.d1i.opt()], outs=[self.d1o.opt()])

    # ---------------- GN1 scalars ----------------
    def post_ar1_fetch(self):
        nc, s = self.nc, self.s
        con = self.p["consts"]
        self.ar1 = con.tile([C, 2], F32, name=f"ar1_{s}")
        nc.gpsimd.dma_start(out=self.ar1[:], in_=_bcast(self.d1o[:], C))

    def post_ar1(self):
        nc, s = self.nc, self.s
        con = self.p["consts"]
        cols = self.p["cols"]
        ar1 = self.ar1
        mu = con.tile([C, 1], F32, name=f"mu1_{s}")
        nc.vector.tensor_scalar_mul(out=mu[:], in0=ar1[:, 0:1],
                                    scalar1=INV_N)
        var = con.tile([C, 1], F32, name=f"v1_{s}")
        nc.vector.tensor_scalar_mul(out=var[:], in0=ar1[:, 1:2],
                                    scalar1=INV_N)
        musq = con.tile([C, 1], F32, name=f"mq1_{s}")
        nc.vector.tensor_mul(out=musq[:], in0=mu[:], in1=mu[:])
        nc.vector.tensor_sub(out=var[:], in0=var[:], in1=musq[:])
        nc.vector.tensor_scalar_add(out=var[:], in0=var[:], scalar1=EPS)
        inv = self._rsqrt(var[:], V1MID, "a")
        self.scale1 = con.tile([C, 1], F32, name=f"sc1_{s}")
        nc.vector.tensor_mul(out=self.scale1[:], in0=inv[:],
                             in1=cols[:, 1:2])
        self.bias1 = con.tile([C, 1], F32, name=f"bi1_{s}")
        nc.vector.tensor_sub(out=self.bias1[:], in0=cols[:, 0:1], in1=mu[:])
        nc.vector.tensor_mul(out=self.bias1[:], in0=self.bias1[:],
                             in1=self.scale1[:])
        nc.vector.tensor_add(out=self.bias1[:], in0=self.bias1[:],
                             in1=cols[:, 2:3])

    def _gn_chunk(self, r0, r1):
        nc = self.nc
        nc.scalar.activation(out=self.xact[0:C, r0:r1, 1:W + 1],
                             in_=self.xact[0:C, r0:r1, 1:W + 1],
                             func=AF.Gelu, bias=self.bias1[:],
                             scale=self.scale1[:])
        if r0 == 0:
            nc.vector.tensor_scalar_mul(out=self.xact[0:C, 0:1, :],
                                        in0=self.xact[0:C, 0:1, :],
                                        scalar1=self.p["em"][:, 2 * self.s:
                                                            2 * self.s + 1])
        if r1 == RF:
            nc.vector.tensor_scalar_mul(
                out=self.xact[0:C, RF - 1:RF, :],
                in0=self.xact[0:C, RF - 1:RF, :],
                scalar1=self.p["em"][:, 2 * self.s + 1:2 * self.s + 2])

    # ---------------- phase B ----------------
    def stage_b_start(self):
        self._gn_r = 0

    def stage_b_pair(self, psb, p):
        nc, s = self.nc, self.s
        cols = self.p["cols"]
        wb = self.p["wb"]
        need = min(4 * p + 6, RF)
        while self._gn_r < need:
            r1 = min(self._gn_r + 8, RF)
            self._gn_chunk(self._gn_r, r1)
            self._gn_r = r1
        ps1 = psb.tile([M, 4, 512], F32, tag="p21")
        ps2 = psb.tile([M, 4, 512], F32, tag="p22")
        for wi, ps, branches in ((0, ps1, (BR_LR, BR_LDIAG)),
                                 (1, ps2, (BR_TD, BR_RDIAG))):
            for slot in range(4):
                br = branches[slot // 2]
                tt = 2 * p + (slot % 2)
                pr = 2 * tt + 1
                for j, (dh, dw) in enumerate(br):
                    if ROWTILE:
                        lhs = wb[32 * j:32 * (j + 1), wi, :]
                        rhs = self.xact[32 * j:32 * (j + 1),
                                        pr + dh:pr + dh + 2,
                                        1 + dw:1 + dw + W]
                    else:
                        lhs = self.p["wbz"][:, 3 * wi + j, :]
                        rhs = self.xact[:, pr + dh:pr + dh + 2,
                                        1 + dw:1 + dw + W]
                    nc.tensor.matmul(out=ps[:, slot, :], lhsT=lhs, rhs=rhs,
                                     start=(j == 0), stop=(j == 2))
        gst = self.p["gst"]
        g1 = gst.tile([C, 4, 512], F16, tag="g1")
        nc.scalar.activation(out=g1[:], in_=ps1[0:C, :, :], func=AF.Gelu,
                             bias=cols[:, 3:4])
        g2 = gst.tile([C, 4, 512], F16, tag="g2")
        nc.scalar.activation(out=g2[:], in_=ps2[0:C, :, :], func=AF.Gelu,
                             bias=cols[:, 4:5])
        tmp = self.p["tmp"]
        ga = tmp.tile([C, 2, 512], F16, tag="ga")
        nc.gpsimd.tensor_add(out=ga[:], in0=g1[:, 0:2, :], in1=g1[:, 2:4, :])
        gb = tmp.tile([C, 2, 512], F16, tag="gb")
        nc.vector.tensor_add(out=gb[:], in0=g2[:, 0:2, :], in1=g2[:, 2:4, :])
        od = self.opre[:, 1024 * p:1024 * (p + 1)].rearrange(
            "p (k w) -> p k w", w=512)
        nc.vector.scalar_tensor_tensor(
            out=od, in0=ga[:], scalar=0.0, in1=gb[:],
            op0=ALU.add, op1=ALU.add, accum_out=self.sodcol[:, p:p + 1])
        jb = tmp.tile([C, 2, 512], F16, tag="jb")
        nc.vector.scalar_tensor_tensor(
            out=jb[:], in0=od, scalar=0.0, in1=od,
            op0=ALU.add, op1=ALU.mult, accum_out=self.sq2col[:, p:p + 1])

    def stage_b_finish(self, psb):
        nc, s = self.nc, self.s
        con = self.p["consts"]
        pack2 = con.tile([C, 2], F32, name=f"pk2_{s}")
        nc.vector.tensor_reduce(out=pack2[:, 0:1], in_=self.sodcol[:],
                                axis=AX.X, op=ALU.add)
        nc.vector.tensor_reduce(out=pack2[:, 1:2], in_=self.sq2col[:],
                                axis=AX.X, op=ALU.add)
        spt = psb.tile([M, 4, 512], F32, tag="p21")
        nc.tensor.matmul(out=spt[0:1, 0, 0:2], lhsT=self.p["ones96"][:],
                         rhs=pack2[:], start=True, stop=True)
        ar_in = con.tile([1, 2], F32, name=f"ar2i_{s}")
        nc.vector.tensor_copy(out=ar_in[:], in_=spt[0:1, 0, 0:2])
        nc.sync.dma_start(out=self.d2i[:], in_=ar_in[:])
        nc.gpsimd.collective_compute(
            "AllReduce", ALU.add, replica_groups=self.groups,
            ins=[self.d2i.opt()], outs=[self.d2o.opt()])

    # ---------------- GN2 scalars + conv3 weight fold ----------------
    def post_ar2_fetch(self):
        nc, s = self.nc, self.s
        con = self.p["consts"]
        self.ar2 = con.tile([C, 2], F32, name=f"ar2_{s}")
        nc.gpsimd.dma_start(out=self.ar2[:], in_=_bcast(self.d2o[:], C))

    def post_ar2(self):
        nc, s = self.nc, self.s
        con = self.p["consts"]
        cols = self.p["cols"]
        ar2 = self.ar2
        mu = con.tile([C, 1], F32, name=f"mu2_{s}")
        nc.vector.tensor_scalar_mul(out=mu[:], in0=ar2[:, 0:1],
                                    scalar1=INV_N)
        var = con.tile([C, 1], F32, name=f"v2_{s}")
        nc.vector.tensor_scalar_mul(out=var[:], in0=ar2[:, 1:2],
                                    scalar1=INV_N)
        musq = con.tile([C, 1], F32, name=f"mq2_{s}")
        nc.vector.tensor_mul(out=musq[:], in0=mu[:], in1=mu[:])
        nc.vector.tensor_sub(out=var[:], in0=var[:], in1=musq[:])
        nc.vector.tensor_scalar_add(out=var[:], in0=var[:], scalar1=EPS)
        inv = self._rsqrt(var[:], V2MID, "b")
        self.w3ts = con.tile([C, M], F16, name=f"w3s_{s}")
        nc.vector.tensor_scalar_mul(out=self.w3ts[:],
                                    in0=self.p["w3gt"][:],
                                    scalar1=inv[:])
        s2 = con.tile([C, 1], F32, name=f"s2c_{s}")
        nc.vector.tensor_mul(out=s2[:], in0=inv[:], in1=mu[:])
        self.ccol = con.tile([C, 1], F32, name=f"cc_{s}")
        nc.vector.tensor_mul(out=self.ccol[:], in0=s2[:], in1=cols[:, 6:7])
        nc.vector.tensor_sub(out=self.ccol[:], in0=cols[:, 5:6],
                             in1=self.ccol[:])

    # ---------------- phase C ----------------
    def stage_c_batch(self, psb, cb):
        nc, s = self.nc, self.s
        ost = self.p["ost"]
        out = self.io["out"][s]
        pc = psb.tile([M, 4, 512], F32, tag=("p21" if cb % 2 == 0
                                               else "p22"))
        for slot in range(4):
            tt = 4 * cb + slot
            if ROWTILE:
                for j in range(3):
                    nc.tensor.matmul(
                        out=pc[:, slot, :],
                        lhsT=self.w3ts[32 * j:32 * (j + 1), :],
                        rhs=self.opre[32 * j:32 * (j + 1),
                                      512 * tt:512 * (tt + 1)],
                        start=(j == 0), stop=(j == 2))
            else:
                nc.tensor.matmul(
                    out=pc[:, slot, :], lhsT=self.w3ts[:],
                    rhs=self.opre[:, 512 * tt:512 * (tt + 1)],
                    start=True, stop=True)
        o = ost.tile([C, 4, 512], F16, tag="o")
        nc.vector.tensor_scalar(out=o[:, 0:2, :], in0=pc[0:C, 0:2, :],
                                scalar1=self.ccol[:], scalar2=None,
                                op0=ALU.add)
        nc.scalar.activation(out=o[:, 2:4, :], in_=pc[0:C, 2:4, :],
                             func=AF.Identity, bias=self.ccol[:])
        nc.sync.dma_start(
            out=out[:, 8 * cb:8 * cb + 8, :].rearrange(
                "p (n r) w -> p n r w", r=2),
            in_=o[:].rearrange("p n (r w) -> p n r w", w=W))


def _emit(nc, tc, ctx, groups, io):
    pools = {
        "consts": ctx.enter_context(tc.tile_pool(name="consts", bufs=1)),
        "big": ctx.enter_context(tc.tile_pool(name="big", bufs=1)),
        "xin": ctx.enter_context(tc.tile_pool(name="xin", bufs=3)),
        "gst": ctx.enter_context(tc.tile_pool(name="gst", bufs=2)),
        "tmp": ctx.enter_context(tc.tile_pool(name="tmp", bufs=2)),
        "ost": ctx.enter_context(tc.tile_pool(name="ost", bufs=3)),
        "dram": ctx.enter_context(tc.tile_pool(name="dram", bufs=1,
                                               space="DRAM")),
    }
    con = pools["consts"]
    w1t = con.tile([C, M], F16)
    nc.sync.dma_start(out=w1t[:], in_=io["w1t"][:])
    wb = con.tile([C, 2, M], F16)
    nc.sync.dma_start(out=wb[:], in_=io["wb"][:])
    w3gt = con.tile([C, M], F32)
    nc.sync.dma_start(out=w3gt[:], in_=io["w3gt"][:])
    cols = con.tile([C, 7], F32)
    nc.sync.dma_start(out=cols[:], in_=io["cols"][:])
    em = con.tile([C, 2 * S], F32)
    nc.gpsimd.dma_start(out=em[:], in_=_bcast(io["em"][:], C))
    ones96 = con.tile([C, 1], F32)
    nc.vector.memset(ones96[:], 1.0)
    zc = con.tile([C, 1, 1, 1], F16)
    nc.vector.memset(zc[:], 0.0)
    w1z = con.tile([C, 3, M], F16)
    nc.sync.dma_start(out=w1z[:], in_=io["w1z"][:])
    wbz = con.tile([C, 6, M], F16)
    nc.sync.dma_start(out=wbz[:], in_=io["wbz"][:])
    pools.update(w1t=w1t, wb=wb, w3gt=w3gt, cols=cols, em=em,
                 ones96=ones96, zc=zc, w1z=w1z, wbz=wbz)

    # warm the Gelu activation table early (overlaps input DMA)
    warm = con.tile([C, 2], F32)
    nc.vector.memset(warm[:], 0.0)
    nc.scalar.activation(out=warm[:], in_=warm[:], func=AF.Gelu,
                         bias=0.0)

    # warm up the collectives firmware path (result unused)
    dw_i = pools["dram"].tile([1, 2], F32)
    dw_o = pools["dram"].tile([1, 2], F32)
    nc.sync.dma_start(out=dw_i[:], in_=warm[0:1, 0:2])
    nc.gpsimd.collective_compute("AllReduce", ALU.add, replica_groups=groups,
                                 ins=[dw_i.opt()], outs=[dw_o.opt()])

    streams = [_Stream(nc, tc, pools, groups, io, s) for s in range(S)]

    s0, s1 = streams
    with tc.tile_pool(name="psa", bufs=2, space="PSUM") as psa:
        s0.stage_a(psa)
        s0.post_ar1_fetch()
        s1.stage_a(psa)
        s1.post_ar1_fetch()
    with tc.tile_pool(name="psb", bufs=1, space="PSUM") as psb:
        s0.post_ar1()
        s0.stage_b_start()
        for p in range(NPAIR):
            s0.stage_b_pair(psb, p)
            if p == 2:
                s1.post_ar1()
        s0.stage_b_finish(psb)
        s0.post_ar2_fetch()
        s1.stage_b_start()
        for p in range(NPAIR):
            s1.stage_b_pair(psb, p)
        s1.stage_b_finish(psb)
        s1.post_ar2_fetch()
        s0.post_ar2()
        for cb in range(8):
            s0.stage_c_batch(psb, cb)
        s1.post_ar2()
        for cb in range(8):
            s1.stage_c_batch(psb, cb)


def build_program(n_cores=N_CORES, n_samples=B, n_streams=S):
    import contextlib
    cps = n_cores * n_streams // n_samples      # cores per sample
    groups = [list(range(a * cps, (a + 1) * cps))
              for a in range(n_cores // cps)]
    nc = bacc.Bacc("TRN2", target_bir_lowering=False, debug=False,
                   enable_asserts=False, num_devices=n_cores)
    io = {
        "xs": nc.dram_tensor("xs", [n_streams, C, RF, W], F16,
                             kind="ExternalInput").ap(),
        "em": nc.dram_tensor("em", [1, 2 * n_streams], F32,
                             kind="ExternalInput").ap(),
        "w1t": nc.dram_tensor("w1t", [C, M], F16, kind="ExternalInput").ap(),
        "wb": nc.dram_tensor("wb", [C, 2 * M], F16,
                             kind="ExternalInput").ap(),
        "w3gt": nc.dram_tensor("w3gt", [C, M], F32,
                               kind="ExternalInput").ap(),
        "w1z": nc.dram_tensor("w1z", [C, 3 * M], F16,
                              kind="ExternalInput").ap(),
        "wbz": nc.dram_tensor("wbz", [C, 6 * M], F16,
                              kind="ExternalInput").ap(),
        "cols": nc.dram_tensor("cols", [C, 7], F32,
                               kind="ExternalInput").ap(),
        "out": nc.dram_tensor("out", [n_streams, C, ROWS_SLAB, W], F16,
                              kind="ExternalOutput").ap(),
    }
    io["wb"] = io["wb"].rearrange("p (k m) -> p k m", m=M)
    io["w1z"] = io["w1z"].rearrange("p (k m) -> p k m", m=M)
    io["wbz"] = io["wbz"].rearrange("p (k m) -> p k m", m=M)
    with tile.TileContext(nc) as tc:
        with contextlib.ExitStack() as ctx:
            _emit(nc, tc, ctx, groups, io)
    nc.compile()
    return nc


def host_inputs(x, w1, b1, w21, b21, w22, b22, w3, b3,
                gn1_w, gn1_b, gn2_w, gn2_b,
                rows_slab=ROWS_SLAB, n_cores=N_CORES, n_streams=S):
    x = np.asarray(x, np.float32)
    nb_, _, hh, _ = x.shape
    cps = n_cores * n_streams // nb_
    w1 = np.asarray(w1, np.float32)
    w21 = np.asarray(w21, np.float32)
    w22 = np.asarray(w22, np.float32)
    w3 = np.asarray(w3, np.float32)

    w1t = np.zeros((C, M), np.float16)
    w1t[:, 0:C] = w1.T
    wb = np.zeros((C, 2 * M), np.float16)
    wb[:, 0:C] = w21.T
    wb[:, M:M + C] = w22.T
    w3gt = np.zeros((C, M), np.float32)
    w3gt[:, 0:C] = (w3 * np.asarray(gn2_w)[None, :]).T
    w1z = np.zeros((C, 3 * M), np.float16)
    wbz = np.zeros((C, 6 * M), np.float16)
    for j in range(3):
        w1z[32 * j:32 * (j + 1), j * M:j * M + C] = \
            w1.T[32 * j:32 * (j + 1), :]
        for wi, wm in enumerate((w21, w22)):
            wbz[32 * j:32 * (j + 1), (3 * wi + j) * M:(3 * wi + j) * M + C] \
                = wm.T[32 * j:32 * (j + 1), :].astype(np.float16)
    shared = {
        "w1t": w1t,
        "wb": wb,
        "w1z": w1z,
        "wbz": wbz,
        "w3gt": w3gt,
        "cols": np.ascontiguousarray(np.stack(
            [np.asarray(b1, np.float32), np.asarray(gn1_w, np.float32),
             np.asarray(gn1_b, np.float32), np.asarray(b21, np.float32),
             np.asarray(b22, np.float32),
             (np.asarray(b3) + w3 @ np.asarray(gn2_b)).astype(np.float32),
             (w3 * np.asarray(gn2_w)[None, :]).sum(1).astype(np.float32)],
            axis=1)),
    }
    x16 = x.astype(np.float16)
    in_maps = []
    for k in range(n_cores):
        xs = np.zeros((n_streams, C, rows_slab + 2, W), np.float16)
        em = np.zeros((1, 2 * n_streams), np.float32)
        for s in range(n_streams):
            bidx = s * (nb_ // n_streams) + k // cps
            q = k % cps
            h0 = q * rows_slab
            lo, hi = h0 - 1, h0 + rows_slab + 1
            slo, shi = max(lo, 0), min(hi, hh)
            xs[s, :, slo - lo:slo - lo + (shi - slo), :] = \
                x16[bidx, :, slo:shi, :]
            em[0, 2 * s] = 1.0 if lo >= 0 else 0.0
            em[0, 2 * s + 1] = 1.0 if hi <= hh else 0.0
        in_maps.append({"xs": xs, "em": em, **shared})
    return in_maps


def gather_output(results, rows_slab=ROWS_SLAB, n_cores=N_CORES,
                  n_streams=S, n_samples=B, hh=H):
    cps = n_cores * n_streams // n_samples
    out = np.empty((n_samples, C, hh, W), np.float32)
    for k in range(n_cores):
        for s in range(n_streams):
            bidx = s * (n_samples // n_streams) + k // cps
            q = k % cps
            out[bidx, :, q * rows_slab:(q + 1) * rows_slab, :] = \
                results[k]["out"][s].astype(np.float32)
    return out


_PROGRAM = None


def kernel(x, w1, b1, w21, b21, w22, b22, w3, b3, gn1_w, gn1_b, gn2_w, gn2_b):
    global _PROGRAM
    from concourse.bass_utils import run_bass_kernel_spmd
    from concourse.bass_interp import get_hw_module
    if _PROGRAM is None:
        nc = build_program()
        nc.m = get_hw_module(nc.m)
        _PROGRAM = nc
    nc = _PROGRAM
    in_maps = host_inputs(x, w1, b1, w21, b21, w22, b22, w3, b3,
                          gn1_w, gn1_b, gn2_w, gn2_b)
    res = run_bass_kernel_spmd(nc, in_maps, core_ids=list(range(N_CORES)))
    return gather_output(res.results)


# revision 21
# speedup vs baseline: 1.1465x; 1.1465x over previous
"""Trainium2 Bass kernel for the AxialShift block (4x96x256x256, fp32).

Self-contained: builds an 8-core SPMD Bass program, compiles it once,
and runs it via run_bass_kernel_spmd.

Sharding: each core runs S=2 streams; stream s of core k handles a
quarter-sample slab (64 rows) of sample 2s + k//4 (4 cores share a
sample; GroupNorm(1 group) partials cross cores via 8-byte AllReduces).

Per-stream pipeline (x read once as fp16, out written once as fp16):
  A   : conv1 as one full-K fp16 matmul per 512-px unit over the 66-row
        halo frame; PSUM evacuation to the fp16 frame fused with the
        per-channel GroupNorm-1 sums via tensor_scalar accum_out
        (DVE/ACT split); sum-of-squares from the fp16 frame via
        scalar_tensor_tensor accum; per-channel partials collapsed with
        a ones-matmul; 8-byte AllReduce (result prefetched on the
        gpsimd queue so the wait does not block other engines).
  B   : GN1 scale/bias + erf-Gelu applied lazily in place (ACT); the 4
        axial-shift branch convs as 12 chunk-masked K=96 matmuls per
        512-px tile, two tiles in flight across all 8 PSUM banks
        (shifts are free-dim offsets into the padded frame); branch
        bias folded into the Gelu activation bias (N=2048 calls);
        branch sums split gpsimd/DVE, branch total + GroupNorm-2
        partials fused via scalar_tensor_tensor accum; second
        AllReduce.
  C   : GroupNorm-2 folded into conv3 weights on device (1/std from a
        DVE Newton rsqrt, avoiding ACT Sqrt table switches); one
        full-K matmul per tile alternating two PSUM rings; bias-add
        drain alternating DVE/ACT; fp16 output DMA.
"""
import sys

sys.path.insert(0, "/opt/trn_rl_repo")

import numpy as np

import concourse.bass as bass
import concourse.bacc as bacc
import concourse.tile as tile
from concourse import mybir

F32 = mybir.dt.float32
F16 = mybir.dt.float16

C = 96
M = 128           # weight free dim padded to 128 (FWL)
H = 256
W = 256
B = 4
WP = W + 2
N_CORES = 8
S = 2             # streams per core
ROWS_SLAB = H * B // (N_CORES * S)       # 64
RF = ROWS_SLAB + 2                       # 66 frame rows
NPAIR = ROWS_SLAB // 4                   # 16 tile-pairs (tile = 2 rows)
EPS = 1e-5
NPX = ROWS_SLAB * W                      # true pixels per stream
INV_N = 1.0 / (C * H * W)                # per-sample GroupNorm 1/N
V1MID = 0.337                            # measured conv1-output variance
V2MID = 0.2363
ROWTILE = False    # PE row-tiled K=32 matmuls (True) vs full-K masked blocks                           # measured branch-sum variance
AF = mybir.ActivationFunctionType
ALU = mybir.AluOpType
AX = mybir.AxisListType

# (dh, dw) read offsets per chunk j=0,1,2 (s_j = -1, 0, +1):
BR_LR = [(0, 1), (0, 0), (0, -1)]
BR_LDIAG = [(1, 1), (0, 0), (-1, -1)]
BR_TD = [(1, 0), (0, 0), (-1, 0)]
BR_RDIAG = [(1, -1), (0, 0), (-1, 1)]


def _bcast(ap, nparts):
    return bass.AP(tensor=ap.tensor, offset=ap.offset,
                   ap=[[0, nparts]] + list(ap.ap[1:]))


def _bcast_free(ap, dims):
    """Broadcast a [C,1] column over extra 0-stride free dims."""
    return bass.AP(tensor=ap.tensor, offset=ap.offset,
                   ap=[list(ap.ap[0])] + [[0, d] for d in dims])


class _Stream:
    """Per-stream state; stages are emitted by the orchestrator."""

    def __init__(self, nc, tc, pools, groups, io, s):
        self.nc, self.tc, self.s = nc, tc, s
        self.p = pools
        self.groups = groups
        self.io = io
        con = pools["consts"]
        big = pools["big"]
        self.xact = big.tile([C, RF, WP], F16, name=f"xact{s}")
        self.opre = big.tile([C, ROWS_SLAB * W], F16, name=f"opre{s}")
        self.scol = con.tile([C, 10], F32, name=f"sc_{s}")
        self.sqcol = con.tile([C, 9], F32, name=f"sq_{s}")
        self.junkc = con.tile([C, 2], F32, name=f"jc_{s}")
        self.sodcol = con.tile([C, NPAIR], F32, name=f"so_{s}")
        self.sq2col = con.tile([C, NPAIR], F32, name=f"s2_{s}")
        dram = pools["dram"]
        self.d1i = dram.tile([1, 2], F32, name=f"d1i{s}")
        self.d1o = dram.tile([1, 2], F32, name=f"d1o{s}")
        self.d2i = dram.tile([1, 2], F32, name=f"d2i{s}")
        self.d2o = dram.tile([1, 2], F32, name=f"d2o{s}")

    # ---------------- helpers ----------------
    def _rsqrt(self, vpe, mid, tag):
        """1/sqrt(vpe) on [C,1] via linear seed + 3 Newton iters (DVE)."""
        nc = self.nc
        con = self.p["consts"]
        rm = mid ** -0.5
        bcoef = -0.4764 * rm / mid
        acoef = 1.5768 * rm
        vh = con.tile([C, 1], F32, name=f"vh{tag}_{self.s}")
        nc.vector.tensor_scalar_mul(out=vh[:], in0=vpe, scalar1=0.5)
        r = con.tile([C, 1], F32, name=f"nr{tag}_{self.s}")
        nc.vector.tensor_scalar(out=r[:], in0=vpe, scalar1=bcoef,
                                scalar2=acoef, op0=ALU.mult, op1=ALU.add)
        u = con.tile([C, 1], F32, name=f"nu{tag}_{self.s}")
        w = con.tile([C, 1], F32, name=f"nw{tag}_{self.s}")
        for _ in range(3):
            nc.vector.tensor_mul(out=u[:], in0=r[:], in1=r[:])
            nc.vector.tensor_mul(out=w[:], in0=u[:], in1=vh[:])
            nc.vector.tensor_scalar(out=w[:], in0=w[:], scalar1=-1.0,
                                    scalar2=1.5, op0=ALU.mult, op1=ALU.add)
            nc.vector.tensor_mul(out=r[:], in0=r[:], in1=w[:])
        return r

    def _evac(self, pt_ap, out_ap, acc_ap, eng="v"):
        """PSUM->fp16 copy fused with per-channel sum (DVE or ACT)."""
        nc = self.nc
        if eng == "s":
            nc.scalar.activation(out=out_ap, in_=pt_ap, func=AF.Copy,
                                 bias=0.0, accum_out=acc_ap)
        else:
            nc.vector.tensor_scalar(out=out_ap, in0=pt_ap, scalar1=0.0,
                                    scalar2=None, op0=ALU.add, op1=ALU.add,
                                    accum_out=acc_ap)

    # ---------------- phase A ----------------
    def stage_a(self, psa):
        nc, s = self.nc, self.s
        xin = self.p["xin"]
        xs = self.io["xs"][s]
        w1t = self.p["w1t"]
        nc.vector.memset(self.xact[:, :, 0:1], 0.0)
        nc.vector.memset(self.xact[:, :, WP - 1:WP], 0.0)
        for b in range(9):
            r0 = 8 * b
            nr = 8 if b < 8 else 2
            xt = xin.tile([C, 8, W], F16, tag="xt")
            nc.sync.dma_start(out=xt[:, 0:nr, :], in_=xs[:, r0:r0 + nr, :])
            pt = psa.tile([M, 4, 512], F32, tag="pta")
            for q in range(nr // 2):
                if ROWTILE:
                    for j in range(3):
                        nc.tensor.matmul(
                            out=pt[:, q, :],
                            lhsT=w1t[32 * j:32 * (j + 1), :],
                            rhs=xt[32 * j:32 * (j + 1), 2 * q:2 * q + 2, :],
                            start=(j == 0), stop=(j == 2))
                else:
                    nc.tensor.matmul(out=pt[:, q, :], lhsT=w1t[:],
                                     rhs=xt[:, 2 * q:2 * q + 2, :],
                                     start=True, stop=True)
            if b == 0:
                self._evac(pt[0:C, 0, 0:256], self.xact[0:C, 0, 1:W + 1],
                           self.junkc[:, 0:1])
                self._evac(pt[0:C, 0, 256:512], self.xact[0:C, 1, 1:W + 1],
                           self.scol[:, 9:10])
                self._evac(
                    pt[0:C, 1:4, :].rearrange("p k (r w) -> p k r w", w=W),
                    self.xact[0:C, 2:8, 1:W + 1].rearrange(
                        "p (k r) w -> p k r w", r=2),
                    self.scol[:, 0:1])
                sq_in = self.xact[0:C, 1:8, 0:WP]
            elif b == 8:
                self._evac(pt[0:C, 0, 0:256], self.xact[0:C, 64, 1:W + 1],
                           self.scol[:, 8:9])
                self._evac(pt[0:C, 0, 256:512], self.xact[0:C, 65, 1:W + 1],
                           self.junkc[:, 1:2])
                sq_in = self.xact[0:C, 64:65, 0:WP]
            else:
                self._evac(
                    pt[0:C, :, :].rearrange("p k (r w) -> p k r w", w=W),
                    self.xact[0:C, r0:r0 + 8, 1:W + 1].rearrange(
                        "p (k r) w -> p k r w", r=2),
                    self.scol[:, b:b + 1],
                    eng=("s" if b in (1, 2, 4, 5, 7) else "v"))
                sq_in = self.xact[0:C, r0:r0 + 8, 0:WP]
            ja = self.p["tmp"].tile([C, 8, WP], F16, tag="ja")
            nc.vector.scalar_tensor_tensor(
                out=ja[:, 0:sq_in.shape[1], :], in0=sq_in, scalar=0.0,
                in1=sq_in, op0=ALU.add, op1=ALU.mult,
                accum_out=self.sqcol[:, b:b + 1])
        # aggregate per-channel partials, fold conv1 bias analytically
        con = self.p["consts"]
        cols = self.p["cols"]
        s1 = con.tile([C, 1], F32, name=f"s1_{s}")
        nc.vector.tensor_reduce(out=s1[:], in_=self.scol[:], axis=AX.X,
                                op=ALU.add)
        s2 = con.tile([C, 1], F32, name=f"s2s_{s}")
        nc.vector.tensor_reduce(out=s2[:], in_=self.sqcol[:], axis=AX.X,
                                op=ALU.add)
        tb = con.tile([C, 1], F32, name=f"tb_{s}")
        nc.vector.tensor_scalar_mul(out=tb[:], in0=cols[:, 0:1],
                                    scalar1=float(NPX))
        pack = con.tile([C, 2], F32, name=f"pk1_{s}")
        nc.vector.tensor_add(out=pack[:, 0:1], in0=s1[:], in1=tb[:])
        q = con.tile([C, 1], F32, name=f"q_{s}")
        nc.vector.tensor_scalar_mul(out=q[:], in0=s1[:], scalar1=2.0)
        nc.vector.tensor_add(out=q[:], in0=q[:], in1=tb[:])
        nc.vector.tensor_mul(out=q[:], in0=q[:], in1=cols[:, 0:1])
        nc.vector.tensor_add(out=pack[:, 1:2], in0=s2[:], in1=q[:])
        spt = psa.tile([M, 4, 512], F32, tag="pta")
        nc.tensor.matmul(out=spt[0:1, 0, 0:2], lhsT=self.p["ones96"][:],
                         rhs=pack[:], start=True, stop=True)
        ar_in = con.tile([1, 2], F32, name=f"ar1i_{s}")
        nc.vector.tensor_copy(out=ar_in[:], in_=spt[0:1, 0, 0:2])
        nc.sync.dma_start(out=self.d1i[:], in_=ar_in[:])
        nc.gpsimd.collective_compute(
            "AllReduce", ALU.add, replica_groups=self.groups,
            ins=[self.d1i.opt()], outs=[self.d1o.opt()])

    # ---------------- GN1 scalars ----------------
    def post_ar1_fetch(self):
        nc, s = self.nc, self.s
        con = self.p["consts"]
        self.ar1 = con.tile([C, 2], F32, name=f"ar1_{s}")
        nc.gpsimd.dma_start(out=self.ar1[:], in_=_bcast(self.d1o[:], C))

    def post_ar1(self):
        nc, s = self.nc, self.s
        con = self.p["consts"]
        cols = self.p["cols"]
        ar1 = self.ar1
        mu = con.tile([C, 1], F32, name=f"mu1_{s}")
        nc.vector.tensor_scalar_mul(out=mu[:], in0=ar1[:, 0:1],
                                    scalar1=INV_N)
        var = con.tile([C, 1], F32, name=f"v1_{s}")
        nc.vector.tensor_scalar_mul(out=var[:], in0=ar1[:, 1:2],
                                    scalar1=INV_N)
        musq = con.tile([C, 1], F32, name=f"mq1_{s}")
        nc.vector.tensor_mul(out=musq[:], in0=mu[:], in1=mu[:])
        nc.vector.tensor_sub(out=var[:], in0=var[:], in1=musq[:])
        nc.vector.tensor_scalar_add(out=var[:], in0=var[:], scalar1=EPS)
        inv = self._rsqrt(var[:], V1MID, "a")
        self.scale1 = con.tile([C, 1], F32, name=f"sc1_{s}")
        nc.vector.tensor_mul(out=self.scale1[:], in0=inv[:],
                             in1=cols[:, 1:2])
        self.bias1 = con.tile([C, 1], F32, name=f"bi1_{s}")
        nc.vector.tensor_sub(out=self.bias1[:], in0=cols[:, 0:1], in1=mu[:])
        nc.vector.tensor_mul(out=self.bias1[:], in0=self.bias1[:],
                             in1=self.scale1[:])
        nc.vector.tensor_add(out=self.bias1[:], in0=self.bias1[:],
                             in1=cols[:, 2:3])

    def _gn_chunk(self, r0, r1):
        nc = self.nc
        nc.scalar.activation(out=self.xact[0:C, r0:r1, 1:W + 1],
                             in_=self.xact[0:C, r0:r1, 1:W + 1],
                             func=AF.Gelu, bias=self.bias1[:],
                             scale=self.scale1[:])
        if r0 == 0:
            nc.vector.tensor_scalar_mul(out=self.xact[0:C, 0:1, :],
                                        in0=self.xact[0:C, 0:1, :],
                                        scalar1=self.p["em"][:, 2 * self.s:
                                                            2 * self.s + 1])
        if r1 == RF:
            nc.vector.tensor_scalar_mul(
                out=self.xact[0:C, RF - 1:RF, :],
                in0=self.xact[0:C, RF - 1:RF, :],
                scalar1=self.p["em"][:, 2 * self.s + 1:2 * self.s + 2])

    # ---------------- phase B ----------------
    def stage_b_start(self):
        self._gn_r = 0

    def stage_b_pair(self, psb, p):
        nc, s = self.nc, self.s
        cols = self.p["cols"]
        wb = self.p["wb"]
        need = min(4 * p + 6, RF)
        while self._gn_r < need:
            r1 = min(self._gn_r + 8, RF)
            self._gn_chunk(self._gn_r, r1)
            self._gn_r = r1
        ps1 = psb.tile([M, 4, 512], F32, tag="p21")
        ps2 = psb.tile([M, 4, 512], F32, tag="p22")
        for wi, ps, branches in ((0, ps1, (BR_LR, BR_LDIAG)),
                                 (1, ps2, (BR_TD, BR_RDIAG))):
            for slot in range(4):
                br = branches[slot // 2]
                tt = 2 * p + (slot % 2)
                pr = 2 * tt + 1
                for j, (dh, dw) in enumerate(br):
                    if ROWTILE:
                        lhs = wb[32 * j:32 * (j + 1), wi, :]
                        rhs = self.xact[32 * j:32 * (j + 1),
                                        pr + dh:pr + dh + 2,
                                        1 + dw:1 + dw + W]
                    else:
                        lhs = self.p["wbz"][:, 3 * wi + j, :]
                        rhs = self.xact[:, pr + dh:pr + dh + 2,
                                        1 + dw:1 + dw + W]
                    nc.tensor.matmul(out=ps[:, slot, :], lhsT=lhs, rhs=rhs,
                                     start=(j == 0), stop=(j == 2))
        gst = self.p["gst"]
        g1 = gst.tile([C, 4, 512], F16, tag="g1")
        nc.scalar.activation(out=g1[:], in_=ps1[0:C, :, :], func=AF.Gelu,
                             bias=cols[:, 3:4])
        g2 = gst.tile([C, 4, 512], F16, tag="g2")
        nc.scalar.activation(out=g2[:], in_=ps2[0:C, :, :], func=AF.Gelu,
                             bias=cols[:, 4:5])
        tmp = self.p["tmp"]
        ga = tmp.tile([C, 2, 512], F16, tag="ga")
        nc.gpsimd.tensor_add(out=ga[:], in0=g1[:, 0:2, :], in1=g1[:, 2:4, :])
        gb = tmp.tile([C, 2, 512], F16, tag="gb")
        nc.vector.tensor_add(out=gb[:], in0=g2[:, 0:2, :], in1=g2[:, 2:4, :])
        od = self.opre[:, 1024 * p:1024 * (p + 1)].rearrange(
            "p (k w) -> p k w", w=512)
        nc.vector.scalar_tensor_tensor(
            out=od, in0=ga[:], scalar=0.0, in1=gb[:],
            op0=ALU.add, op1=ALU.add, accum_out=self.sodcol[:, p:p + 1])
        jb = tmp.tile([C, 2, 512], F16, tag="jb")
        nc.vector.scalar_tensor_tensor(
            out=jb[:], in0=od, scalar=0.0, in1=od,
            op0=ALU.add, op1=ALU.mult, accum_out=self.sq2col[:, p:p + 1])

    def stage_b_finish(self, psb):
        nc, s = self.nc, self.s
        con = self.p["consts"]
        pack2 = con.tile([C, 2], F32, name=f"pk2_{s}")
        nc.vector.tensor_reduce(out=pack2[:, 0:1], in_=self.sodcol[:],
                                axis=AX.X, op=ALU.add)
        nc.vector.tensor_reduce(out=pack2[:, 1:2], in_=self.sq2col[:],
                                axis=AX.X, op=ALU.add)
        spt = psb.tile([M, 4, 512], F32, tag="p21")
        nc.tensor.matmul(out=spt[0:1, 0, 0:2], lhsT=self.p["ones96"][:],
                         rhs=pack2[:], start=True, stop=True)
        ar_in = con.tile([1, 2], F32, name=f"ar2i_{s}")
        nc.vector.tensor_copy(out=ar_in[:], in_=spt[0:1, 0, 0:2])
        nc.sync.dma_start(out=self.d2i[:], in_=ar_in[:])
        nc.gpsimd.collective_compute(
            "AllReduce", ALU.add, replica_groups=self.groups,
            ins=[self.d2i.opt()], outs=[self.d2o.opt()])

    # ---------------- GN2 scalars + conv3 weight fold ----------------
    def post_ar2_fetch(self):
        nc, s = self.nc, self.s
        con = self.p["consts"]
        self.ar2 = con.tile([C, 2], F32, name=f"ar2_{s}")
        nc.gpsimd.dma_start(out=self.ar2[:], in_=_bcast(self.d2o[:], C))

    def post_ar2(self):
        nc, s = self.nc, self.s
        con = self.p["consts"]
        cols = self.p["cols"]
        ar2 = self.ar2
        mu = con.tile([C, 1], F32, name=f"mu2_{s}")
        nc.vector.tensor_scalar_mul(out=mu[:], in0=ar2[:, 0:1],
                                    scalar1=INV_N)
        var = con.tile([C, 1], F32, name=f"v2_{s}")
        nc.vector.tensor_scalar_mul(out=var[:], in0=ar2[:, 1:2],
                                    scalar1=INV_N)
        musq = con.tile([C, 1], F32, name=f"mq2_{s}")
        nc.vector.tensor_mul(out=musq[:], in0=mu[:], in1=mu[:])
        nc.vector.tensor_sub(out=var[:], in0=var[:], in1=musq[:])
        nc.vector.tensor_scalar_add(out=var[:], in0=var[:], scalar1=EPS)
        inv = self._rsqrt(var[:], V2MID, "b")
        self.w3ts = con.tile([C, M], F16, name=f"w3s_{s}")
        nc.vector.tensor_scalar_mul(out=self.w3ts[:],
                                    in0=self.p["w3gt"][:],
                                    scalar1=inv[:])
        s2 = con.tile([C, 1], F32, name=f"s2c_{s}")
        nc.vector.tensor_mul(out=s2[:], in0=inv[:], in1=mu[:])
        self.ccol = con.tile([C, 1], F32, name=f"cc_{s}")
        nc.vector.tensor_mul(out=self.ccol[:], in0=s2[:], in1=cols[:, 6:7])
        nc.vector.tensor_sub(out=self.ccol[:], in0=cols[:, 5:6],
                             in1=self.ccol[:])

    # ---------------- phase C ----------------
    def stage_c_batch(self, psb, cb):
        nc, s = self.nc, self.s
        ost = self.p["ost"]
        out = self.io["out"][s]
        pc = psb.tile([M, 4, 512], F32, tag=("p21" if cb % 2 == 0
                                               else "p22"))
        for slot in range(4):
            tt = 4 * cb + slot
            if ROWTILE:
                for j in range(3):
                    nc.tensor.matmul(
                        out=pc[:, slot, :],
                        lhsT=self.w3ts[32 * j:32 * (j + 1), :],
                        rhs=self.opre[32 * j:32 * (j + 1),
                                      512 * tt:512 * (tt + 1)],
                        start=(j == 0), stop=(j == 2))
            else:
                nc.tensor.matmul(
                    out=pc[:, slot, :], lhsT=self.w3ts[:],
                    rhs=self.opre[:, 512 * tt:512 * (tt + 1)],
                    start=True, stop=True)
        o = ost.tile([C, 4, 512], F16, tag="o")
        if cb % 2 == 0:
            nc.vector.tensor_scalar(out=o[:], in0=pc[0:C, :, :],
                                    scalar1=self.ccol[:], scalar2=None,
                                    op0=ALU.add)
        else:
            nc.scalar.activation(out=o[:], in_=pc[0:C, :, :],
                                 func=AF.Identity, bias=self.ccol[:])
        nc.sync.dma_start(
            out=out[:, 8 * cb:8 * cb + 8, :].rearrange(
                "p (n r) w -> p n r w", r=2),
            in_=o[:].rearrange("p n (r w) -> p n r w", w=W))


def _emit(nc, tc, ctx, groups, io):
    pools = {
        "consts": ctx.enter_context(tc.tile_pool(name="consts", bufs=1)),
        "big": ctx.enter_context(tc.tile_pool(name="big", bufs=1)),
        "xin": ctx.enter_context(tc.tile_pool(name="xin", bufs=3)),
        "gst": ctx.enter_context(tc.tile_pool(name="gst", bufs=2)),
        "tmp": ctx.enter_context(tc.tile_pool(name="tmp", bufs=2)),
        "ost": ctx.enter_context(tc.tile_pool(name="ost", bufs=3)),
        "dram": ctx.enter_context(tc.tile_pool(name="dram", bufs=1,
                                               space="DRAM")),
    }
    con = pools["consts"]
    w1t = con.tile([C, M], F16)
    nc.sync.dma_start(out=w1t[:], in_=io["w1t"][:])
    wb = con.tile([C, 2, M], F16)
    nc.sync.dma_start(out=wb[:], in_=io["wb"][:])
    w3gt = con.tile([C, M], F32)
    cols = con.tile([C, 7], F32)
    nc.sync.dma_start(out=cols[:], in_=io["cols"][:])
    em = con.tile([C, 2 * S], F32)
    nc.gpsimd.dma_start(out=em[:], in_=_bcast(io["em"][:], C))
    ones96 = con.tile([C, 1], F32)
    nc.vector.memset(ones96[:], 1.0)
    zc = con.tile([C, 1, 1, 1], F16)
    nc.vector.memset(zc[:], 0.0)
    wbz = con.tile([C, 6, M], F16)
    pools.update(w1t=w1t, wb=wb, w3gt=w3gt, cols=cols, em=em,
                 ones96=ones96, zc=zc, wbz=wbz)

    # warm the Gelu activation table early (overlaps input DMA)
    warm = con.tile([C, 2], F32)
    nc.vector.memset(warm[:], 0.0)
    nc.scalar.activation(out=warm[:], in_=warm[:], func=AF.Gelu,
                         bias=0.0)

    # warm up the collectives firmware path (result unused)
    dw_i = pools["dram"].tile([1, 2], F32)
    dw_o = pools["dram"].tile([1, 2], F32)
    nc.sync.dma_start(out=dw_i[:], in_=warm[0:1, 0:2])
    nc.gpsimd.collective_compute("AllReduce", ALU.add, replica_groups=groups,
                                 ins=[dw_i.opt()], outs=[dw_o.opt()])

    streams = [_Stream(nc, tc, pools, groups, io, s) for s in range(S)]

    s0, s1 = streams
    with tc.tile_pool(name="psa", bufs=2, space="PSUM") as psa:
        s0.stage_a(psa)
        s0.post_ar1_fetch()
        # branch/conv3 weights aren't needed until phase B / post-AR2;
        # load them after stream 0's input rows have queue priority
        nc.sync.dma_start(out=pools["wbz"][:], in_=io["wbz"][:])
        nc.sync.dma_start(out=pools["w3gt"][:], in_=io["w3gt"][:])
        s1.stage_a(psa)
        s1.post_ar1_fetch()
    with tc.tile_pool(name="psb", bufs=1, space="PSUM") as psb:
        s0.post_ar1()
        s0.stage_b_start()
        for p in range(NPAIR):
            s0.stage_b_pair(psb, p)
            if p == 2:
                s1.post_ar1()
        s0.stage_b_finish(psb)
        s0.post_ar2_fetch()
        s1.stage_b_start()
        for p in range(NPAIR):
            s1.stage_b_pair(psb, p)
        s1.stage_b_finish(psb)
        s1.post_ar2_fetch()
        s0.post_ar2()
        for cb in range(8):
            s0.stage_c_batch(psb, cb)
        s1.post_ar2()
        for cb in range(8):
            s1.stage_c_batch(psb, cb)


def build_program(n_cores=N_CORES, n_samples=B, n_streams=S):
    import contextlib
    cps = n_cores * n_streams // n_samples      # cores per sample
    groups = [list(range(a * cps, (a + 1) * cps))
              for a in range(n_cores // cps)]
    nc = bacc.Bacc("TRN2", target_bir_lowering=False, debug=False,
                   enable_asserts=False, num_devices=n_cores)
    io = {
        "xs": nc.dram_tensor("xs", [n_streams, C, RF, W], F16,
                             kind="ExternalInput").ap(),
        "em": nc.dram_tensor("em", [1, 2 * n_streams], F32,
                             kind="ExternalInput").ap(),
        "w1t": nc.dram_tensor("w1t", [C, M], F16, kind="ExternalInput").ap(),
        "wb": nc.dram_tensor("wb", [C, 2 * M], F16,
                             kind="ExternalInput").ap(),
        "w3gt": nc.dram_tensor("w3gt", [C, M], F32,
                               kind="ExternalInput").ap(),
        "w1z": nc.dram_tensor("w1z", [C, 3 * M], F16,
                              kind="ExternalInput").ap(),
        "wbz": nc.dram_tensor("wbz", [C, 6 * M], F16,
                              kind="ExternalInput").ap(),
        "cols": nc.dram_tensor("cols", [C, 7], F32,
                               kind="ExternalInput").ap(),
        "out": nc.dram_tensor("out", [n_streams, C, ROWS_SLAB, W], F16,
                              kind="ExternalOutput").ap(),
    }
    io["wb"] = io["wb"].rearrange("p (k m) -> p k m", m=M)
    io["w1z"] = io["w1z"].rearrange("p (k m) -> p k m", m=M)
    io["wbz"] = io["wbz"].rearrange("p (k m) -> p k m", m=M)
    with tile.TileContext(nc) as tc:
        with contextlib.ExitStack() as ctx:
            _emit(nc, tc, ctx, groups, io)
    nc.compile()
    return nc


def host_inputs(x, w1, b1, w21, b21, w22, b22, w3, b3,
                gn1_w, gn1_b, gn2_w, gn2_b,
                rows_slab=ROWS_SLAB, n_cores=N_CORES, n_streams=S):
    x = np.asarray(x, np.float32)
    nb_, _, hh, _ = x.shape
    cps = n_cores * n_streams // nb_
    w1 = np.asarray(w1, np.float32)
    w21 = np.asarray(w21, np.float32)
    w22 = np.asarray(w22, np.float32)
    w3 = np.asarray(w3, np.float32)

    w1t = np.zeros((C, M), np.float16)
    w1t[:, 0:C] = w1.T
    wb = np.zeros((C, 2 * M), np.float16)
    wb[:, 0:C] = w21.T
    wb[:, M:M + C] = w22.T
    w3gt = np.zeros((C, M), np.float32)
    w3gt[:, 0:C] = (w3 * np.asarray(gn2_w)[None, :]).T
    w1z = np.zeros((C, 3 * M), np.float16)
    wbz = np.zeros((C, 6 * M), np.float16)
    for j in range(3):
        w1z[32 * j:32 * (j + 1), j * M:j * M + C] = \
            w1.T[32 * j:32 * (j + 1), :]
        for wi, wm in enumerate((w21, w22)):
            wbz[32 * j:32 * (j + 1), (3 * wi + j) * M:(3 * wi + j) * M + C] \
                = wm.T[32 * j:32 * (j + 1), :].astype(np.float16)
    shared = {
        "w1t": w1t,
        "wb": wb,
        "w1z": w1z,
        "wbz": wbz,
        "w3gt": w3gt,
        "cols": np.ascontiguousarray(np.stack(
            [np.asarray(b1, np.float32), np.asarray(gn1_w, np.float32),
             np.asarray(gn1_b, np.float32), np.asarray(b21, np.float32),
             np.asarray(b22, np.float32),
             (np.asarray(b3) + w3 @ np.asarray(gn2_b)).astype(np.float32),
             (w3 * np.asarray(gn2_w)[None, :]).sum(1).astype(np.float32)],
            axis=1)),
    }
    x16 = x.astype(np.float16)
    in_maps = []
    for k in range(n_cores):
        xs = np.zeros((n_streams, C, rows_slab + 2, W), np.float16)
        em = np.zeros((1, 2 * n_streams), np.float32)
        for s in range(n_streams):
            bidx = s * (nb_ // n_streams) + k // cps
            q = k % cps
            h0 = q * rows_slab
            lo, hi = h0 - 1, h0 + rows_slab + 1
            slo, shi = max(lo, 0), min(hi, hh)
            xs[s, :, slo - lo:slo - lo + (shi - slo), :] = \
                x16[bidx, :, slo:shi, :]
            em[0, 2 * s] = 1.0 if lo >= 0 else 0.0
            em[0, 2 * s + 1] = 1.0 if hi <= hh else 0.0
        in_maps.append({"xs": xs, "em": em, **shared})
    return in_maps


def gather_output(results, rows_slab=ROWS_SLAB, n_cores=N_CORES,
                  n_streams=S, n_samples=B, hh=H):
    cps = n_cores * n_streams // n_samples
    out = np.empty((n_samples, C, hh, W), np.float32)
    for k in range(n_cores):
        for s in range(n_streams):
            bidx = s * (n_samples // n_streams) + k // cps
            q = k % cps
            out[bidx, :, q * rows_slab:(q + 1) * rows_slab, :] = \
                results[k]["out"][s].astype(np.float32)
    return out


_PROGRAM = None


def kernel(x, w1, b1, w21, b21, w22, b22, w3, b3, gn1_w, gn1_b, gn2_w, gn2_b):
    global _PROGRAM
    from concourse.bass_utils import run_bass_kernel_spmd
    from concourse.bass_interp import get_hw_module
    if _PROGRAM is None:
        nc = build_program()
        nc.m = get_hw_module(nc.m)
        _PROGRAM = nc
    nc = _PROGRAM
    in_maps = host_inputs(x, w1, b1, w21, b21, w22, b22, w3, b3,
                          gn1_w, gn1_b, gn2_w, gn2_b)
    res = run_bass_kernel_spmd(nc, in_maps, core_ids=list(range(N_CORES)))
    return gather_output(res.results)


# revision 22
# speedup vs baseline: 1.1682x; 1.0190x over previous
"""Trainium2 Bass kernel for the AxialShift block (4x96x256x256, fp32).

Self-contained: builds an 8-core SPMD Bass program, compiles it once,
and runs it via run_bass_kernel_spmd.

Sharding: each core runs S=2 streams; stream s of core k handles a
quarter-sample slab (64 rows) of sample 2s + k//4 (4 cores share a
sample; GroupNorm(1 group) partials cross cores via 8-byte AllReduces).

Per-stream pipeline (x read once as fp16, out written once as fp16):
  A   : conv1 as one full-K fp16 matmul per 512-px unit over the 66-row
        halo frame; PSUM evacuation to the fp16 frame fused with the
        per-channel GroupNorm-1 sums via tensor_scalar accum_out
        (DVE/ACT split); sum-of-squares from the fp16 frame via
        scalar_tensor_tensor accum; per-channel partials collapsed with
        a ones-matmul; 8-byte AllReduce (result prefetched on the
        gpsimd queue so the wait does not block other engines).
  B   : GN1 scale/bias + erf-Gelu applied lazily in place (ACT); the 4
        axial-shift branch convs as 12 chunk-masked K=96 matmuls per
        512-px tile, two tiles in flight across all 8 PSUM banks
        (shifts are free-dim offsets into the padded frame); branch
        bias folded into the Gelu activation bias (N=2048 calls);
        branch sums split gpsimd/DVE, branch total + GroupNorm-2
        partials fused via scalar_tensor_tensor accum; second
        AllReduce.
  C   : GroupNorm-2 folded into conv3 weights on device (1/std from a
        DVE Newton rsqrt, avoiding ACT Sqrt table switches); one
        full-K matmul per tile alternating two PSUM rings; bias-add
        drain alternating DVE/ACT; fp16 output DMA.
"""
import sys

sys.path.insert(0, "/opt/trn_rl_repo")

import numpy as np

import concourse.bass as bass
import concourse.bacc as bacc
import concourse.tile as tile
from concourse import mybir

F32 = mybir.dt.float32
F16 = mybir.dt.float16

C = 96
M = 128           # weight free dim padded to 128 (FWL)
H = 256
W = 256
B = 4
WP = W + 2
N_CORES = 8
S = 2             # streams per core
ROWS_SLAB = H * B // (N_CORES * S)       # 64
RF = ROWS_SLAB + 2                       # 66 frame rows
NPAIR = ROWS_SLAB // 4                   # 16 tile-pairs (tile = 2 rows)
EPS = 1e-5
NPX = ROWS_SLAB * W                      # true pixels per stream
INV_N = 1.0 / (C * H * W)                # per-sample GroupNorm 1/N
V1MID = 0.337                            # measured conv1-output variance
V2MID = 0.2363
ROWTILE = False    # PE row-tiled K=32 matmuls (True) vs full-K masked blocks                           # measured branch-sum variance
AF = mybir.ActivationFunctionType
ALU = mybir.AluOpType
AX = mybir.AxisListType

# (dh, dw) read offsets per chunk j=0,1,2 (s_j = -1, 0, +1):
BR_LR = [(0, 1), (0, 0), (0, -1)]
BR_LDIAG = [(1, 1), (0, 0), (-1, -1)]
BR_TD = [(1, 0), (0, 0), (-1, 0)]
BR_RDIAG = [(1, -1), (0, 0), (-1, 1)]


def _bcast(ap, nparts):
    return bass.AP(tensor=ap.tensor, offset=ap.offset,
                   ap=[[0, nparts]] + list(ap.ap[1:]))


def _bcast_free(ap, dims):
    """Broadcast a [C,1] column over extra 0-stride free dims."""
    return bass.AP(tensor=ap.tensor, offset=ap.offset,
                   ap=[list(ap.ap[0])] + [[0, d] for d in dims])


class _Stream:
    """Per-stream state; stages are emitted by the orchestrator."""

    def __init__(self, nc, tc, pools, groups, io, s):
        self.nc, self.tc, self.s = nc, tc, s
        self.p = pools
        self.groups = groups
        self.io = io
        con = pools["consts"]
        big = pools["big"]
        self.xact = big.tile([C, RF, WP], F16, name=f"xact{s}")
        self.opre = big.tile([C, ROWS_SLAB * W], F16, name=f"opre{s}")
        self.scol = con.tile([C, 10], F32, name=f"sc_{s}")
        self.sqcol = con.tile([C, 9], F32, name=f"sq_{s}")
        self.junkc = con.tile([C, 2], F32, name=f"jc_{s}")
        self.sodcol = con.tile([C, NPAIR], F32, name=f"so_{s}")
        self.sq2col = con.tile([C, NPAIR], F32, name=f"s2_{s}")
        dram = pools["dram"]
        self.d1i = dram.tile([1, 2], F32, name=f"d1i{s}")
        self.d1o = dram.tile([1, 2], F32, name=f"d1o{s}")
        self.d2i = dram.tile([1, 2], F32, name=f"d2i{s}")
        self.d2o = dram.tile([1, 2], F32, name=f"d2o{s}")

    # ---------------- helpers ----------------
    def _rsqrt(self, vpe, mid, tag):
        """1/sqrt(vpe) on [C,1] via linear seed + 3 Newton iters (DVE)."""
        nc = self.nc
        con = self.p["consts"]
        rm = mid ** -0.5
        bcoef = -0.4764 * rm / mid
        acoef = 1.5768 * rm
        vh = con.tile([C, 1], F32, name=f"vh{tag}_{self.s}")
        nc.vector.tensor_scalar_mul(out=vh[:], in0=vpe, scalar1=0.5)
        r = con.tile([C, 1], F32, name=f"nr{tag}_{self.s}")
        nc.vector.tensor_scalar(out=r[:], in0=vpe, scalar1=bcoef,
                                scalar2=acoef, op0=ALU.mult, op1=ALU.add)
        u = con.tile([C, 1], F32, name=f"nu{tag}_{self.s}")
        w = con.tile([C, 1], F32, name=f"nw{tag}_{self.s}")
        for _ in range(3):
            nc.vector.tensor_mul(out=u[:], in0=r[:], in1=r[:])
            nc.vector.tensor_mul(out=w[:], in0=u[:], in1=vh[:])
            nc.vector.tensor_scalar(out=w[:], in0=w[:], scalar1=-1.0,
                                    scalar2=1.5, op0=ALU.mult, op1=ALU.add)
            nc.vector.tensor_mul(out=r[:], in0=r[:], in1=w[:])
        return r

    def _evac(self, pt_ap, out_ap, acc_ap, eng="v"):
        """PSUM->fp16 copy fused with per-channel sum (DVE or ACT)."""
        nc = self.nc
        if eng == "s":
            nc.scalar.activation(out=out_ap, in_=pt_ap, func=AF.Copy,
                                 bias=0.0, accum_out=acc_ap)
        else:
            nc.vector.tensor_scalar(out=out_ap, in0=pt_ap, scalar1=0.0,
                                    scalar2=None, op0=ALU.add, op1=ALU.add,
                                    accum_out=acc_ap)

    # ---------------- phase A ----------------
    def stage_a(self, psa):
        nc, s = self.nc, self.s
        xin = self.p["xin"]
        xs = self.io["xs"][s]
        w1t = self.p["w1t"]
        nc.vector.memset(self.xact[:, :, 0:1], 0.0)
        nc.vector.memset(self.xact[:, :, WP - 1:WP], 0.0)
        for b in range(9):
            r0 = 8 * b
            nr = 8 if b < 8 else 2
            xt = xin.tile([C, 8, W], F16, tag="xt")
            nc.sync.dma_start(out=xt[:, 0:nr, :], in_=xs[:, r0:r0 + nr, :])
            pt = psa.tile([M, 4, 512], F32, tag="pta")
            for q in range(nr // 2):
                if ROWTILE:
                    for j in range(3):
                        nc.tensor.matmul(
                            out=pt[:, q, :],
                            lhsT=w1t[32 * j:32 * (j + 1), :],
                            rhs=xt[32 * j:32 * (j + 1), 2 * q:2 * q + 2, :],
                            start=(j == 0), stop=(j == 2))
                else:
                    nc.tensor.matmul(out=pt[:, q, :], lhsT=w1t[:],
                                     rhs=xt[:, 2 * q:2 * q + 2, :],
                                     start=True, stop=True)
            if b == 0:
                self._evac(pt[0:C, 0, 0:256], self.xact[0:C, 0, 1:W + 1],
                           self.junkc[:, 0:1])
                self._evac(pt[0:C, 0, 256:512], self.xact[0:C, 1, 1:W + 1],
                           self.scol[:, 9:10])
                self._evac(
                    pt[0:C, 1:4, :].rearrange("p k (r w) -> p k r w", w=W),
                    self.xact[0:C, 2:8, 1:W + 1].rearrange(
                        "p (k r) w -> p k r w", r=2),
                    self.scol[:, 0:1])
                sq_in = self.xact[0:C, 1:8, 0:WP]
            elif b == 8:
                self._evac(pt[0:C, 0, 0:256], self.xact[0:C, 64, 1:W + 1],
                           self.scol[:, 8:9])
                self._evac(pt[0:C, 0, 256:512], self.xact[0:C, 65, 1:W + 1],
                           self.junkc[:, 1:2])
                sq_in = self.xact[0:C, 64:65, 0:WP]
            else:
                self._evac(
                    pt[0:C, :, :].rearrange("p k (r w) -> p k r w", w=W),
                    self.xact[0:C, r0:r0 + 8, 1:W + 1].rearrange(
                        "p (k r) w -> p k r w", r=2),
                    self.scol[:, b:b + 1],
                    eng=("s" if b in (1, 2, 4, 5, 7) else "v"))
                sq_in = self.xact[0:C, r0:r0 + 8, 0:WP]
            ja = self.p["tmp"].tile([C, 8, WP], F16, tag="ja")
            nc.vector.scalar_tensor_tensor(
                out=ja[:, 0:sq_in.shape[1], :], in0=sq_in, scalar=0.0,
                in1=sq_in, op0=ALU.add, op1=ALU.mult,
                accum_out=self.sqcol[:, b:b + 1])
        # aggregate per-channel partials, fold conv1 bias analytically
        con = self.p["consts"]
        cols = self.p["cols"]
        s1 = con.tile([C, 1], F32, name=f"s1_{s}")
        nc.vector.tensor_reduce(out=s1[:], in_=self.scol[:], axis=AX.X,
                                op=ALU.add)
        s2 = con.tile([C, 1], F32, name=f"s2s_{s}")
        nc.vector.tensor_reduce(out=s2[:], in_=self.sqcol[:], axis=AX.X,
                                op=ALU.add)
        tb = con.tile([C, 1], F32, name=f"tb_{s}")
        nc.vector.tensor_scalar_mul(out=tb[:], in0=cols[:, 0:1],
                                    scalar1=float(NPX))
        pack = con.tile([C, 2], F32, name=f"pk1_{s}")
        nc.vector.tensor_add(out=pack[:, 0:1], in0=s1[:], in1=tb[:])
        q = con.tile([C, 1], F32, name=f"q_{s}")
        nc.vector.tensor_scalar_mul(out=q[:], in0=s1[:], scalar1=2.0)
        nc.vector.tensor_add(out=q[:], in0=q[:], in1=tb[:])
        nc.vector.tensor_mul(out=q[:], in0=q[:], in1=cols[:, 0:1])
        nc.vector.tensor_add(out=pack[:, 1:2], in0=s2[:], in1=q[:])
        spt = psa.tile([M, 4, 512], F32, tag="pta")
        nc.tensor.matmul(out=spt[0:1, 0, 0:2], lhsT=self.p["ones96"][:],
                         rhs=pack[:], start=True, stop=True)
        ar_in = con.tile([1, 2], F32, name=f"ar1i_{s}")
        nc.vector.tensor_copy(out=ar_in[:], in_=spt[0:1, 0, 0:2])
        nc.sync.dma_start(out=self.d1i[:], in_=ar_in[:])
        nc.gpsimd.collective_compute(
            "AllReduce", ALU.add, replica_groups=self.groups,
            ins=[self.d1i.opt()], outs=[self.d1o.opt()])

    # ---------------- GN1 scalars ----------------
    def post_ar1_fetch(self):
        nc, s = self.nc, self.s
        con = self.p["consts"]
        self.ar1 = con.tile([C, 2], F32, name=f"ar1_{s}")
        nc.gpsimd.dma_start(out=self.ar1[:], in_=_bcast(self.d1o[:], C))

    def post_ar1(self):
        nc, s = self.nc, self.s
        con = self.p["consts"]
        cols = self.p["cols"]
        ar1 = self.ar1
        mu = con.tile([C, 1], F32, name=f"mu1_{s}")
        nc.vector.tensor_scalar_mul(out=mu[:], in0=ar1[:, 0:1],
                                    scalar1=INV_N)
        var = con.tile([C, 1], F32, name=f"v1_{s}")
        nc.vector.tensor_scalar_mul(out=var[:], in0=ar1[:, 1:2],
                                    scalar1=INV_N)
        musq = con.tile([C, 1], F32, name=f"mq1_{s}")
        nc.vector.tensor_mul(out=musq[:], in0=mu[:], in1=mu[:])
        nc.vector.tensor_sub(out=var[:], in0=var[:], in1=musq[:])
        nc.vector.tensor_scalar_add(out=var[:], in0=var[:], scalar1=EPS)
        inv = self._rsqrt(var[:], V1MID, "a")
        self.scale1 = con.tile([C, 1], F32, name=f"sc1_{s}")
        nc.vector.tensor_mul(out=self.scale1[:], in0=inv[:],
                             in1=cols[:, 1:2])
        self.bias1 = con.tile([C, 1], F32, name=f"bi1_{s}")
        nc.vector.tensor_sub(out=self.bias1[:], in0=cols[:, 0:1], in1=mu[:])
        nc.vector.tensor_mul(out=self.bias1[:], in0=self.bias1[:],
                             in1=self.scale1[:])
        nc.vector.tensor_add(out=self.bias1[:], in0=self.bias1[:],
                             in1=cols[:, 2:3])

    def _gn_chunk(self, r0, r1):
        nc = self.nc
        nc.scalar.activation(out=self.xact[0:C, r0:r1, 1:W + 1],
                             in_=self.xact[0:C, r0:r1, 1:W + 1],
                             func=AF.Gelu, bias=self.bias1[:],
                             scale=self.scale1[:])
        if r0 == 0:
            nc.vector.tensor_scalar_mul(out=self.xact[0:C, 0:1, :],
                                        in0=self.xact[0:C, 0:1, :],
                                        scalar1=self.p["em"][:, 2 * self.s:
                                                            2 * self.s + 1])
        if r1 == RF:
            nc.vector.tensor_scalar_mul(
                out=self.xact[0:C, RF - 1:RF, :],
                in0=self.xact[0:C, RF - 1:RF, :],
                scalar1=self.p["em"][:, 2 * self.s + 1:2 * self.s + 2])

    # ---------------- phase B ----------------
    def stage_b_start(self):
        self._gn_r = 0

    def stage_b_pair(self, psb, p):
        nc, s = self.nc, self.s
        cols = self.p["cols"]
        wb = self.p["wb"]
        need = min(4 * p + 6, RF)
        while self._gn_r < need:
            r1 = min(self._gn_r + 8, RF)
            self._gn_chunk(self._gn_r, r1)
            self._gn_r = r1
        ps1 = psb.tile([M, 4, 512], F32, tag="p21")
        ps2 = psb.tile([M, 4, 512], F32, tag="p22")
        for wi, ps, branches in ((0, ps1, (BR_LR, BR_LDIAG)),
                                 (1, ps2, (BR_TD, BR_RDIAG))):
            for slot in range(4):
                br = branches[slot // 2]
                tt = 2 * p + (slot % 2)
                pr = 2 * tt + 1
                for j, (dh, dw) in enumerate(br):
                    if ROWTILE:
                        lhs = wb[32 * j:32 * (j + 1), wi, :]
                        rhs = self.xact[32 * j:32 * (j + 1),
                                        pr + dh:pr + dh + 2,
                                        1 + dw:1 + dw + W]
                    else:
                        lhs = self.p["wbz"][:, 3 * wi + j, :]
                        rhs = self.xact[:, pr + dh:pr + dh + 2,
                                        1 + dw:1 + dw + W]
                    nc.tensor.matmul(out=ps[:, slot, :], lhsT=lhs, rhs=rhs,
                                     start=(j == 0), stop=(j == 2))
        gst = self.p["gst"]
        g1 = gst.tile([C, 4, 512], F16, tag="g1")
        nc.scalar.activation(out=g1[:], in_=ps1[0:C, :, :], func=AF.Gelu,
                             bias=cols[:, 3:4])
        g2 = gst.tile([C, 4, 512], F16, tag="g2")
        nc.scalar.activation(out=g2[:], in_=ps2[0:C, :, :], func=AF.Gelu,
                             bias=cols[:, 4:5])
        tmp = self.p["tmp"]
        ga = tmp.tile([C, 2, 512], F16, tag="ga")
        nc.gpsimd.tensor_add(out=ga[:], in0=g1[:, 0:2, :], in1=g1[:, 2:4, :])
        gb = tmp.tile([C, 2, 512], F16, tag="gb")
        nc.vector.tensor_add(out=gb[:], in0=g2[:, 0:2, :], in1=g2[:, 2:4, :])
        od = self.opre[:, 1024 * p:1024 * (p + 1)].rearrange(
            "p (k w) -> p k w", w=512)
        nc.vector.scalar_tensor_tensor(
            out=od, in0=ga[:], scalar=0.0, in1=gb[:],
            op0=ALU.add, op1=ALU.add, accum_out=self.sodcol[:, p:p + 1])
        jb = tmp.tile([C, 2, 512], F16, tag="jb")
        nc.vector.scalar_tensor_tensor(
            out=jb[:], in0=od, scalar=0.0, in1=od,
            op0=ALU.add, op1=ALU.mult, accum_out=self.sq2col[:, p:p + 1])

    def stage_b_finish(self, psb):
        nc, s = self.nc, self.s
        con = self.p["consts"]
        pack2 = con.tile([C, 2], F32, name=f"pk2_{s}")
        nc.vector.tensor_reduce(out=pack2[:, 0:1], in_=self.sodcol[:],
                                axis=AX.X, op=ALU.add)
        nc.vector.tensor_reduce(out=pack2[:, 1:2], in_=self.sq2col[:],
                                axis=AX.X, op=ALU.add)
        spt = psb.tile([M, 4, 512], F32, tag="p21")
        nc.tensor.matmul(out=spt[0:1, 0, 0:2], lhsT=self.p["ones96"][:],
                         rhs=pack2[:], start=True, stop=True)
        ar_in = con.tile([1, 2], F32, name=f"ar2i_{s}")
        nc.vector.tensor_copy(out=ar_in[:], in_=spt[0:1, 0, 0:2])
        nc.sync.dma_start(out=self.d2i[:], in_=ar_in[:])
        nc.gpsimd.collective_compute(
            "AllReduce", ALU.add, replica_groups=self.groups,
            ins=[self.d2i.opt()], outs=[self.d2o.opt()])

    # ---------------- GN2 scalars + conv3 weight fold ----------------
    def post_ar2_fetch(self):
        nc, s = self.nc, self.s
        con = self.p["consts"]
        self.ar2 = con.tile([C, 2], F32, name=f"ar2_{s}")
        nc.gpsimd.dma_start(out=self.ar2[:], in_=_bcast(self.d2o[:], C))

    def post_ar2(self):
        nc, s = self.nc, self.s
        con = self.p["consts"]
        cols = self.p["cols"]
        ar2 = self.ar2
        mu = con.tile([C, 1], F32, name=f"mu2_{s}")
        nc.vector.tensor_scalar_mul(out=mu[:], in0=ar2[:, 0:1],
                                    scalar1=INV_N)
        var = con.tile([C, 1], F32, name=f"v2_{s}")
        nc.vector.tensor_scalar_mul(out=var[:], in0=ar2[:, 1:2],
                                    scalar1=INV_N)
        musq = con.tile([C, 1], F32, name=f"mq2_{s}")
        nc.vector.tensor_mul(out=musq[:], in0=mu[:], in1=mu[:])
        nc.vector.tensor_sub(out=var[:], in0=var[:], in1=musq[:])
        nc.vector.tensor_scalar_add(out=var[:], in0=var[:], scalar1=EPS)
        inv = self._rsqrt(var[:], V2MID, "b")
        self.w3ts = con.tile([C, M], F16, name=f"w3s_{s}")
        nc.vector.tensor_scalar_mul(out=self.w3ts[:],
                                    in0=self.p["w3gt"][:],
                                    scalar1=inv[:])
        s2 = con.tile([C, 1], F32, name=f"s2c_{s}")
        nc.vector.tensor_mul(out=s2[:], in0=inv[:], in1=mu[:])
        self.ccol = con.tile([C, 1], F32, name=f"cc_{s}")
        nc.vector.tensor_mul(out=self.ccol[:], in0=s2[:], in1=cols[:, 6:7])
        nc.vector.tensor_sub(out=self.ccol[:], in0=cols[:, 5:6],
                             in1=self.ccol[:])

    # ---------------- phase C ----------------
    def stage_c_batch(self, psb, cb):
        nc, s = self.nc, self.s
        ost = self.p["ost"]
        out = self.io["out"][s]
        pc = psb.tile([M, 4, 512], F32, tag=("p21" if cb % 2 == 0
                                               else "p22"))
        for slot in range(4):
            tt = 4 * cb + slot
            if ROWTILE:
                for j in range(3):
                    nc.tensor.matmul(
                        out=pc[:, slot, :],
                        lhsT=self.w3ts[32 * j:32 * (j + 1), :],
                        rhs=self.opre[32 * j:32 * (j + 1),
                                      512 * tt:512 * (tt + 1)],
                        start=(j == 0), stop=(j == 2))
            else:
                nc.tensor.matmul(
                    out=pc[:, slot, :], lhsT=self.w3ts[:],
                    rhs=self.opre[:, 512 * tt:512 * (tt + 1)],
                    start=True, stop=True)
        o = ost.tile([C, 4, 512], F16, tag="o")
        if cb % 2 == 0:
            nc.vector.tensor_scalar(out=o[:], in0=pc[0:C, :, :],
                                    scalar1=self.ccol[:], scalar2=None,
                                    op0=ALU.add)
        else:
            nc.scalar.activation(out=o[:], in_=pc[0:C, :, :],
                                 func=AF.Identity, bias=self.ccol[:])
        nc.sync.dma_start(
            out=out[:, 8 * cb:8 * cb + 8, :].rearrange(
                "p (n r) w -> p n r w", r=2),
            in_=o[:].rearrange("p n (r w) -> p n r w", w=W))


def _emit(nc, tc, ctx, groups, io):
    pools = {
        "consts": ctx.enter_context(tc.tile_pool(name="consts", bufs=1)),
        "big": ctx.enter_context(tc.tile_pool(name="big", bufs=1)),
        "xin": ctx.enter_context(tc.tile_pool(name="xin", bufs=3)),
        "gst": ctx.enter_context(tc.tile_pool(name="gst", bufs=2)),
        "tmp": ctx.enter_context(tc.tile_pool(name="tmp", bufs=2)),
        "ost": ctx.enter_context(tc.tile_pool(name="ost", bufs=3)),
        "dram": ctx.enter_context(tc.tile_pool(name="dram", bufs=1,
                                               space="DRAM")),
    }
    con = pools["consts"]
    w1t = con.tile([C, M], F16)
    nc.sync.dma_start(out=w1t[:], in_=io["w1t"][:])
    wb = con.tile([C, 2, M], F16)
    if ROWTILE:
        nc.sync.dma_start(out=wb[:], in_=io["wb"][:])
    w3gt = con.tile([C, M], F32)
    cols = con.tile([C, 7], F32)
    nc.sync.dma_start(out=cols[:], in_=io["cols"][:])
    em = con.tile([C, 2 * S], F32)
    nc.gpsimd.dma_start(out=em[:], in_=_bcast(io["em"][:], C))
    ones96 = con.tile([C, 1], F32)
    nc.vector.memset(ones96[:], 1.0)
    zc = con.tile([C, 1, 1, 1], F16)
    nc.vector.memset(zc[:], 0.0)
    wbz = con.tile([C, 6, M], F16)
    pools.update(w1t=w1t, wb=wb, w3gt=w3gt, cols=cols, em=em,
                 ones96=ones96, zc=zc, wbz=wbz)

    # warm the Gelu activation table early (overlaps input DMA)
    warm = con.tile([C, 2], F32)
    nc.vector.memset(warm[:], 0.0)
    nc.scalar.activation(out=warm[:], in_=warm[:], func=AF.Gelu,
                         bias=0.0)

    # warm up the collectives firmware path (result unused)
    dw_i = pools["dram"].tile([1, 2], F32)
    dw_o = pools["dram"].tile([1, 2], F32)
    nc.sync.dma_start(out=dw_i[:], in_=warm[0:1, 0:2])
    nc.gpsimd.collective_compute("AllReduce", ALU.add, replica_groups=groups,
                                 ins=[dw_i.opt()], outs=[dw_o.opt()])

    streams = [_Stream(nc, tc, pools, groups, io, s) for s in range(S)]

    s0, s1 = streams
    with tc.tile_pool(name="psa", bufs=2, space="PSUM") as psa:
        s0.stage_a(psa)
        s0.post_ar1_fetch()
        # branch/conv3 weights aren't needed until phase B / post-AR2;
        # load them after stream 0's input rows have queue priority
        nc.sync.dma_start(out=pools["wbz"][:], in_=io["wbz"][:])
        nc.sync.dma_start(out=pools["w3gt"][:], in_=io["w3gt"][:])
        s1.stage_a(psa)
        s1.post_ar1_fetch()
    with tc.tile_pool(name="psb", bufs=1, space="PSUM") as psb:
        s0.post_ar1()
        s0.stage_b_start()
        for p in range(NPAIR):
            s0.stage_b_pair(psb, p)
            if p == 2:
                s1.post_ar1()
        s0.stage_b_finish(psb)
        s0.post_ar2_fetch()
        s1.stage_b_start()
        for p in range(NPAIR):
            s1.stage_b_pair(psb, p)
        s1.stage_b_finish(psb)
        s1.post_ar2_fetch()
        s0.post_ar2()
        for cb in range(8):
            s0.stage_c_batch(psb, cb)
        s1.post_ar2()
        for cb in range(8):
            s1.stage_c_batch(psb, cb)


def build_program(n_cores=N_CORES, n_samples=B, n_streams=S):
    import contextlib
    cps = n_cores * n_streams // n_samples      # cores per sample
    groups = [list(range(a * cps, (a + 1) * cps))
              for a in range(n_cores // cps)]
    nc = bacc.Bacc("TRN2", target_bir_lowering=False, debug=False,
                   enable_asserts=False, num_devices=n_cores)
    io = {
        "xs": nc.dram_tensor("xs", [n_streams, C, RF, W], F16,
                             kind="ExternalInput").ap(),
        "em": nc.dram_tensor("em", [1, 2 * n_streams], F32,
                             kind="ExternalInput").ap(),
        "w1t": nc.dram_tensor("w1t", [C, M], F16, kind="ExternalInput").ap(),
        "wb": nc.dram_tensor("wb", [C, 2 * M], F16,
                             kind="ExternalInput").ap(),
        "w3gt": nc.dram_tensor("w3gt", [C, M], F32,
                               kind="ExternalInput").ap(),
        "w1z": nc.dram_tensor("w1z", [C, 3 * M], F16,
                              kind="ExternalInput").ap(),
        "wbz": nc.dram_tensor("wbz", [C, 6 * M], F16,
                              kind="ExternalInput").ap(),
        "cols": nc.dram_tensor("cols", [C, 7], F32,
                               kind="ExternalInput").ap(),
        "out": nc.dram_tensor("out", [n_streams, C, ROWS_SLAB, W], F16,
                              kind="ExternalOutput").ap(),
    }
    io["wb"] = io["wb"].rearrange("p (k m) -> p k m", m=M)
    io["w1z"] = io["w1z"].rearrange("p (k m) -> p k m", m=M)
    io["wbz"] = io["wbz"].rearrange("p (k m) -> p k m", m=M)
    with tile.TileContext(nc) as tc:
        with contextlib.ExitStack() as ctx:
            _emit(nc, tc, ctx, groups, io)
    nc.compile()
    return nc


def host_inputs(x, w1, b1, w21, b21, w22, b22, w3, b3,
                gn1_w, gn1_b, gn2_w, gn2_b,
                rows_slab=ROWS_SLAB, n_cores=N_CORES, n_streams=S):
    x = np.asarray(x, np.float32)
    nb_, _, hh, _ = x.shape
    cps = n_cores * n_streams // nb_
    w1 = np.asarray(w1, np.float32)
    w21 = np.asarray(w21, np.float32)
    w22 = np.asarray(w22, np.float32)
    w3 = np.asarray(w3, np.float32)

    w1t = np.zeros((C, M), np.float16)
    w1t[:, 0:C] = w1.T
    wb = np.zeros((C, 2 * M), np.float16)
    wb[:, 0:C] = w21.T
    wb[:, M:M + C] = w22.T
    w3gt = np.zeros((C, M), np.float32)
    w3gt[:, 0:C] = (w3 * np.asarray(gn2_w)[None, :]).T
    w1z = np.zeros((C, 3 * M), np.float16)
    wbz = np.zeros((C, 6 * M), np.float16)
    for j in range(3):
        w1z[32 * j:32 * (j + 1), j * M:j * M + C] = \
            w1.T[32 * j:32 * (j + 1), :]
        for wi, wm in enumerate((w21, w22)):
            wbz[32 * j:32 * (j + 1), (3 * wi + j) * M:(3 * wi + j) * M + C] \
                = wm.T[32 * j:32 * (j + 1), :].astype(np.float16)
    shared = {
        "w1t": w1t,
        "wb": wb,
        "w1z": w1z,
        "wbz": wbz,
        "w3gt": w3gt,
        "cols": np.ascontiguousarray(np.stack(
            [np.asarray(b1, np.float32), np.asarray(gn1_w, np.float32),
             np.asarray(gn1_b, np.float32), np.asarray(b21, np.float32),
             np.asarray(b22, np.float32),
             (np.asarray(b3) + w3 @ np.asarray(gn2_b)).astype(np.float32),
             (w3 * np.asarray(gn2_w)[None, :]).sum(1).astype(np.float32)],
            axis=1)),
    }
    x16 = x.astype(np.float16)
    in_maps = []
    for k in range(n_cores):
        xs = np.zeros((n_streams, C, rows_slab + 2, W), np.float16)
        em = np.zeros((1, 2 * n_streams), np.float32)
        for s in range(n_streams):
            bidx = s * (nb_ // n_streams) + k // cps
            q = k % cps
            h0 = q * rows_slab
            lo, hi = h0 - 1, h0 + rows_slab + 1
            slo, shi = max(lo, 0), min(hi, hh)
            xs[s, :, slo - lo:slo - lo + (shi - slo), :] = \
                x16[bidx, :, slo:shi, :]
            em[0, 2 * s] = 1.0 if lo >= 0 else 0.0
            em[0, 2 * s + 1] = 1.0 if hi <= hh else 0.0
        in_maps.append({"xs": xs, "em": em, **shared})
    return in_maps


def gather_output(results, rows_slab=ROWS_SLAB, n_cores=N_CORES,
                  n_streams=S, n_samples=B, hh=H):
    cps = n_cores * n_streams // n_samples
    out = np.empty((n_samples, C, hh, W), np.float32)
    for k in range(n_cores):
        for s in range(n_streams):
            bidx = s * (n_samples // n_streams) + k // cps
            q = k % cps
            out[bidx, :, q * rows_slab:(q + 1) * rows_slab, :] = \
                results[k]["out"][s].astype(np.float32)
    return out


_PROGRAM = None


def kernel(x, w1, b1, w21, b21, w22, b22, w3, b3, gn1_w, gn1_b, gn2_w, gn2_b):
    global _PROGRAM
    from concourse.bass_utils import run_bass_kernel_spmd
    from concourse.bass_interp import get_hw_module
    if _PROGRAM is None:
        nc = build_program()
        nc.m = get_hw_module(nc.m)
        _PROGRAM = nc
    nc = _PROGRAM
    in_maps = host_inputs(x, w1, b1, w21, b21, w22, b22, w3, b3,
                          gn1_w, gn1_b, gn2_w, gn2_b)
    res = run_bass_kernel_spmd(nc, in_maps, core_ids=list(range(N_CORES)))
    return gather_output(res.results)
